# revision 19
# baseline (speedup 1.0000x reference)
"""PointsToBEV Trainium2 kernel.

Sharding: frame b = core//2; each core of a pair owns half the BEV grid
(buckets [0,8192) even cores, [8192,16384) odd). The host routes each point
to the core owning its bucket, so no cross-core reduction is needed.

Scatter strategy: gpsimd.dma_scatter_add loses updates when two descriptors
for the same destination row are in flight concurrently (measured on HW), but
is exact when indices are unique within an instruction and instructions are
serialized via their DMA-completion edge. The host therefore assigns each
bucket's tokens to distinct "rounds" (one scatter instruction each). Rounds
alternate between two parity-split accumulators (bucket%2) so consecutive
rounds touch disjoint tensors and their transfers overlap, while same-buffer
rounds serialize through Tile's WAW edge. Pad slots hold sentinel points that
the device routes to the dummy row (4096); invalid points are dropped on the
host since the reference discards their bucket entirely.

Device pipeline per round: 2-layer point MLP on PE (biases folded via
constant-1 feature rows; count folded as emb channel 80), DVE relu-copy into
the scatter source, one dma_scatter_add of 81-float rows at 512B stride into
acc_E/acc_O [4097, 128] (row 4096 = dummy for invalid/out-of-half points).

Phase 2 per 128-bucket tile (64 even rows stacked over 64 odd rows):
mean = sums * recip(max(cnt,1)), PE transpose, 1x1 conv as matmul with bias
row, BN+ReLU via ACT scale/bias, DMA out. The host un-permutes the stacked
column order when assembling the final (B, 128, 128, 128) output.
"""

import numpy as np

BEV_H, BEV_W = 128, 128
HW = BEV_H * BEV_W
HALF = HW // 2              # buckets per core
QROW = HALF // 2            # rows per parity accumulator (4096)
X_MIN, Y_MIN = -50.0, -50.0
MX = np.float32(0.78125)
B, NP_, C, COUT = 4, 200000, 80, 128
NCORES = 8

# round schedule: (slots, parity) — shared by host and device.
# Invalid points are dropped on the host (they cannot affect the output);
# pad slots use sentinel points that the device maps to the dummy row.
N_BIG, S_BIG = 36, 2560
N_TAIL, S_TAIL = 24, 128
ROUNDS = [(S_BIG, j % 2) for j in range(N_BIG)] + \
         [(S_TAIL, j % 2) for j in range(N_TAIL)]
ROUND_OFF = np.concatenate([[0], np.cumsum([s for s, _ in ROUNDS])])
TOK = int(ROUND_OFF[-1])    # 95232
NW = TOK // 128             # 960
NIX = TOK // 16             # 7680

ES = C + 1                  # 81 floats per scattered row
ESTEP = 128                 # row stride (512B, multiple of 256B)
NROW = QROW + 1             # 4097 rows per accumulator (last = dummy)

_PROG_CACHE = {}


def _f32(x):
    return np.ascontiguousarray(x, dtype=np.float32)


def _build_program(dbg=False):
    from concourse import bacc, mybir, tile
    from concourse.masks import make_identity

    fp32 = mybir.dt.float32
    i16 = mybir.dt.int16
    i32 = mybir.dt.int32

    nc = bacc.Bacc(
        None,
        target_bir_lowering=False,
        debug=False,
        num_devices=NCORES,
        num_swdge_queues=4,
    )

    fp32r = mybir.dt.float32r
    ptsb = nc.dram_tensor("ptsb", [5 * TOK], fp32r, kind="ExternalInput")
    xw_d = nc.dram_tensor("xw", [128, NW], fp32, kind="ExternalInput")
    yw_d = nc.dram_tensor("yw", [128, NW], fp32, kind="ExternalInput")
    hb_d = nc.dram_tensor("hbase", [128, 1], fp32, kind="ExternalInput")
    w1_d = nc.dram_tensor("w1ext", [5, ES], fp32r, kind="ExternalInput")
    w2_d = nc.dram_tensor("w2ext", [ES, ES], fp32, kind="ExternalInput")
    wp_d = nc.dram_tensor("wpext", [ES, COUT], fp32, kind="ExternalInput")
    bnsc_d = nc.dram_tensor("bnsc", [128, 1], fp32, kind="ExternalInput")
    bnsh_d = nc.dram_tensor("bnsh", [128, 1], fp32, kind="ExternalInput")
    outp = nc.dram_tensor("out", [COUT, HALF], fp32, kind="ExternalOutput")
    ackind = "ExternalOutput" if dbg else "Internal"
    acc = [nc.dram_tensor(f"acc{p}", [NROW, ESTEP], fp32, kind=ackind)
           for p in (0, 1)]
    dbg_idx = (nc.dram_tensor("dbgidx", [128, NIX], i16, kind="ExternalOutput")
               if dbg else None)

    with tile.TileContext(nc) as tc:
        with tc.tile_pool(name="consts", bufs=1) as cp:
            w1t = cp.tile([5, ES], fp32r)
            nc.sync.dma_start(out=w1t[:], in_=w1_d[:])
            w2t = cp.tile([ES, ES], fp32)
            nc.sync.dma_start(out=w2t[:], in_=w2_d[:])
            wpt = cp.tile([ES, COUT], fp32)
            nc.sync.dma_start(out=wpt[:], in_=wp_d[:])
            bnsc = cp.tile([128, 1], fp32)
            nc.sync.dma_start(out=bnsc[:], in_=bnsc_d[:])
            bnsh = cp.tile([128, 1], fp32)
            nc.sync.dma_start(out=bnsh[:], in_=bnsh_d[:])
            hbt = cp.tile([128, 1], fp32)
            nc.sync.dma_start(out=hbt[:], in_=hb_d[:])
            ident = cp.tile([128, 128], fp32)
            make_identity(nc, ident[:])
            idxrep = cp.tile([128, NIX], i16)

            with tc.tile_pool(name="idxp", bufs=1) as ip:
                # zero both accumulators (rows [0:4096]; dummy row skipped)
                zt = ip.tile([128, 2048], fp32)
                nc.vector.memset(zt[:], 0.0)
                for p in (0, 1):
                    for k in range(2):
                        nc.sync.dma_start(
                            out=acc[p][k * 2048:(k + 1) * 2048, :], in_=zt[:]
                        )

                # ---- index computation (whole core, [128, NW]) ----
                xwt = ip.tile([128, NW], fp32)
                nc.sync.dma_start(out=xwt[:], in_=xw_d[:])
                ywt = ip.tile([128, NW], fp32)
                nc.sync.dma_start(out=ywt[:], in_=yw_d[:])

                # q = (u - MIN) * fl(1/MX); host pre-nudges points where this
                # disagrees with the reference's IEEE division.
                INV_MX = float(np.float32(1.0) / MX)
                xq = ip.tile([128, NW], fp32)
                nc.vector.tensor_scalar(
                    xq[:], xwt[:], -X_MIN, INV_MX,
                    op0=mybir.AluOpType.add, op1=mybir.AluOpType.mult,
                )
                yq = ip.tile([128, NW], fp32)
                nc.vector.tensor_scalar(
                    yq[:], ywt[:], -Y_MIN, INV_MX,
                    op0=mybir.AluOpType.add, op1=mybir.AluOpType.mult,
                )

                # floor(q) exactly, independent of int-cast rounding mode:
                # k0 = cast(q) within +-1 of floor; k1 = k0 + (q >= k0+1);
                # k = k1 - (q < k1).
                ti = ip.tile([128, NW], i32)
                tp1 = ip.tile([128, NW], fp32)
                ta = ip.tile([128, NW], fp32)

                def floor_exact(out_f, q_ap):
                    nc.vector.tensor_copy(ti[:], q_ap)
                    nc.vector.tensor_copy(out_f[:], ti[:])
                    nc.vector.tensor_scalar(
                        tp1[:], out_f[:], 1.0, None, op0=mybir.AluOpType.add
                    )
                    nc.vector.tensor_tensor(
                        out=ta[:], in0=q_ap, in1=tp1[:],
                        op=mybir.AluOpType.is_ge,
                    )
                    nc.vector.tensor_tensor(
                        out=out_f[:], in0=out_f[:], in1=ta[:],
                        op=mybir.AluOpType.add,
                    )
                    nc.vector.tensor_tensor(
                        out=ta[:], in0=q_ap, in1=out_f[:],
                        op=mybir.AluOpType.is_lt,
                    )
                    nc.vector.tensor_tensor(
                        out=out_f[:], in0=out_f[:], in1=ta[:],
                        op=mybir.AluOpType.subtract,
                    )

                ixf = ip.tile([128, NW], fp32)
                floor_exact(ixf, xq[:])
                iyf = ip.tile([128, NW], fp32)
                floor_exact(iyf, yq[:])

                # g_local = iy*128 + ix - halfbase
                g = ip.tile([128, NW], fp32)
                nc.vector.tensor_scalar(
                    g[:], iyf[:], 128.0, None, op0=mybir.AluOpType.mult
                )
                nc.vector.tensor_tensor(
                    out=g[:], in0=g[:], in1=ixf[:], op=mybir.AluOpType.add
                )
                nc.vector.tensor_scalar(
                    g[:], g[:], hbt[:, 0:1], None, op0=mybir.AluOpType.subtract
                )
                # validity: in-grid and in-half
                v = ip.tile([128, NW], fp32)
                t2 = ip.tile([128, NW], fp32)
                nc.vector.tensor_scalar(
                    v[:], xq[:], 0.0, None, op0=mybir.AluOpType.is_ge
                )
                for src_ap, thr, op in (
                    (xq, 128.0, mybir.AluOpType.is_lt),
                    (yq, 0.0, mybir.AluOpType.is_ge),
                    (yq, 128.0, mybir.AluOpType.is_lt),
                    (g, 0.0, mybir.AluOpType.is_ge),
                    (g, float(HALF), mybir.AluOpType.is_lt),
                ):
                    nc.vector.tensor_scalar(
                        t2[:], src_ap[:], thr, None, op0=op
                    )
                    nc.vector.tensor_tensor(
                        out=v[:], in0=v[:], in1=t2[:], op=mybir.AluOpType.mult
                    )
                # select: g = (g - HALF)*v + HALF  (invalid -> dummy 8192)
                nc.vector.tensor_scalar(
                    g[:], g[:], float(HALF), None, op0=mybir.AluOpType.subtract
                )
                nc.vector.tensor_tensor(
                    out=g[:], in0=g[:], in1=v[:], op=mybir.AluOpType.mult
                )
                nc.vector.tensor_scalar(
                    g[:], g[:], float(HALF), None, op0=mybir.AluOpType.add
                )
                # row = floor(g * 0.5) in [0, 4096]; 4096 = dummy.
                # NB: must not pass tp1 as q_ap — floor_exact writes tp1.
                gh = ip.tile([128, NW], fp32)
                nc.vector.tensor_scalar(
                    gh[:], g[:], 0.5, None, op0=mybir.AluOpType.mult
                )
                row = ip.tile([128, NW], fp32)
                floor_exact(row, gh[:])
                # int16 cast (aligned), then shuffle to the idx wrap via
                # SBUF->SBUF DMA (engine ops need 32-aligned partition bases)
                g16 = ip.tile([128, NW], i16)
                nc.vector.tensor_copy(g16[:], row[:])
                for q in range(8):
                    nc.sync.dma_start(
                        out=idxrep[0:16, q:NIX:8],
                        in_=g16[16 * q:16 * q + 16, :],
                    )
                nc.sync.dma_start(out=idxrep[16:32, :], in_=idxrep[0:16, :])
                nc.sync.dma_start(out=idxrep[32:64, :], in_=idxrep[0:32, :])
                nc.sync.dma_start(out=idxrep[64:128, :], in_=idxrep[0:64, :])
                if dbg:
                    nc.sync.dma_start(out=dbg_idx[:, :], in_=idxrep[:])

            # ---- phase 1: MLP + scatter rounds ----
            with (
                tc.tile_pool(name="pts", bufs=3) as pp,
                tc.tile_pool(name="ht", bufs=3) as hp,
                tc.tile_pool(name="src", bufs=2) as sp,
                tc.tile_pool(name="ps1", bufs=2, space="PSUM") as ps1,
                tc.tile_pool(name="ps2", bufs=3, space="PSUM") as ps2,
            ):
                for rp in range(0, len(ROUNDS), 2):
                  pair = [(rj, ROUNDS[rj]) for rj in (rp, rp + 1)
                          if rj < len(ROUNDS)]
                  off0 = int(ROUND_OFF[rp])
                  stot = sum(s_ for _, (s_, _) in pair)
                  ptst = pp.tile([5, 2 * S_BIG], fp32r, tag="pts")
                  nc.sync.dma_start(
                      out=ptst[:, 0:stot],
                      in_=ptsb[5 * off0:5 * (off0 + stot)],
                  )
                  for rj, (S, par) in pair:
                    off = int(ROUND_OFF[rj])
                    rel = off - off0
                    srct = sp.tile([128, (S_BIG // 128) * ES], fp32, tag="src")
                    done = 0
                    while done < S:
                        # up to 1024 tokens per group: 2 mm1 matmuls into one
                        # 2-bank psum tile, ONE relu, 8 mm2, 2 relu-copies
                        gt = min(1024, S - done)
                        p1 = ps1.tile([ES, 1024], fp32, tag="p1")
                        for h in range(0, gt, 512):
                            nt = min(512, gt - h)
                            nc.tensor.matmul(
                                p1[:, h:h + nt],
                                lhsT=w1t[:],
                                rhs=ptst[:, rel + done + h:rel + done + h + nt],
                                start=True, stop=True,
                            )
                        ht = hp.tile([ES, 1024], fp32, tag="ht")
                        nc.scalar.activation(
                            ht[:, 0:gt], p1[:, 0:gt],
                            mybir.ActivationFunctionType.Relu,
                        )
                        for half in range(0, gt, 512):
                            nt = min(512, gt - half)
                            p2 = ps2.tile([128, 4 * ES], fp32, tag="p2")
                            nm = nt // 128
                            for m in range(nm):
                                nc.tensor.matmul(
                                    p2[:, m * ES:(m + 1) * ES],
                                    lhsT=ht[:, half + m * 128:
                                            half + (m + 1) * 128],
                                    rhs=w2t[:],
                                    start=True, stop=True,
                                )
                            nc.vector.tensor_scalar_max(
                                srct[:, ((done + half) // 128) * ES:
                                     ((done + half) // 128 + nm) * ES],
                                p2[:, 0:nm * ES], 0.0,
                            )
                        done += gt
                    nc.gpsimd.dma_scatter_add(
                        out_ap=acc[par][:, 0:ES],
                        in_ap=srct[:, 0:(S // 128) * ES].rearrange(
                            "p (a b) -> p a b", b=ES
                        ),
                        idxs_ap=idxrep[:, off // 16:off // 16 + S // 16],
                        num_idxs=S,
                        num_idxs_reg=S,
                        elem_size=ES,
                        elem_step=ESTEP,
                        queue_num=rj % 4,
                    )

            # ---- phase 2: grouped 4-tile DMAs ----
            with (
                tc.tile_pool(name="p2s", bufs=3) as p2s,
                tc.tile_pool(name="p2m", bufs=3) as p2m,
                tc.tile_pool(name="pst", bufs=2, space="PSUM") as pst,
                tc.tile_pool(name="pso", bufs=2, space="PSUM") as pso,
            ):
                GT = 4                      # tiles per DMA group
                for tg in range(HALF // 128 // GT):
                    sbG = p2s.tile([128, GT, ESTEP], fp32, tag="sbG")
                    # bucket 2k   -> acc_E row k -> partitions 0..63
                    # bucket 2k+1 -> acc_O row k -> partitions 64..127
                    for par in (0, 1):
                        nc.sync.dma_start(
                            out=sbG[par * 64:par * 64 + 64, :, :],
                            in_=acc[par][tg * GT * 64:(tg + 1) * GT * 64, :]
                            .rearrange("(j p) c -> p j c", p=64),
                        )
                    ob4 = p2s.tile([COUT, GT * 128], fp32, tag="ob4")
                    for j in range(GT):
                        cm = p2m.tile([128, 1], fp32, tag="cm")
                        rc = p2m.tile([128, 1], fp32, tag="rc")
                        mt = p2m.tile([128, ES], fp32, tag="mt")
                        nc.vector.tensor_scalar_max(
                            cm[:], sbG[:, j, C:C + 1], 1.0
                        )
                        nc.vector.reciprocal(rc[:], cm[:])
                        nc.vector.tensor_scalar(
                            mt[:], sbG[:, j, 0:ES], rc[:, 0:1], None,
                            op0=mybir.AluOpType.mult,
                        )
                        nc.vector.memset(mt[:, C:C + 1], 1.0)
                        pt_ = pst.tile([ES, 128], fp32, tag="pt")
                        nc.tensor.transpose(pt_[:], mt[:], ident[:])
                        mtT = p2m.tile([ES, 128], fp32, tag="mtT")
                        nc.scalar.copy(mtT[:], pt_[:])
                        po = pso.tile([COUT, 128], fp32, tag="po")
                        nc.tensor.matmul(
                            po[:], lhsT=wpt[:], rhs=mtT[:],
                            start=True, stop=True,
                        )
                        nc.scalar.activation(
                            ob4[:, j * 128:(j + 1) * 128], po[:],
                            mybir.ActivationFunctionType.Relu,
                            bias=bnsh[:, 0:1], scale=bnsc[:, 0:1],
                        )
                    nc.sync.dma_start(
                        out=outp[:, tg * GT * 128:(tg + 1) * GT * 128],
                        in_=ob4[:],
                    )

    nc.compile()
    return nc


def _assign_rounds(row, parity, is_valid):
    """Assign each token to a round; same (parity,row) never repeats within a
    round. Returns per-token round id. row/parity only meaningful for valid."""
    n = row.shape[0]
    rid = np.empty(n, dtype=np.int64)
    big = [[j for j in range(N_BIG) if j % 2 == p] for p in (0, 1)]
    tail = [[N_BIG + j for j in range(N_TAIL) if j % 2 == p] for p in (0, 1)]
    # invalid tokens: dummy row duplicates are harmless -> spread round-robin
    inv_idx = np.nonzero(~is_valid)[0]
    rid[inv_idx] = np.arange(inv_idx.shape[0]) % N_BIG
    for p in (0, 1):
        sel = is_valid & (parity == p)
        idx = np.nonzero(sel)[0]
        if idx.size == 0:
            continue
        r = row[idx]
        order = np.argsort(r, kind="stable")
        rs = r[order]
        # occurrence number within bucket
        first = np.concatenate([[0], np.nonzero(np.diff(rs))[0] + 1])
        starts = np.zeros(rs.shape[0], dtype=np.int64)
        starts[first] = 1
        grp = np.cumsum(starts) - 1          # bucket enumeration id
        occ = np.arange(rs.shape[0]) - first[grp]
        nb, nt = len(big[p]), len(tail[p])
        assert (occ < nb + nt).all(), "bucket count exceeds round budget"
        rr = np.empty(rs.shape[0], dtype=np.int64)
        lo = occ < nb
        rr[lo] = np.array(big[p])[(grp[lo] + occ[lo]) % nb]
        if (~lo).any():
            rr[~lo] = np.array(tail[p])[(grp[~lo] + occ[~lo]) % nt]
        rid[idx[order]] = rr
    return rid


def _host_prep(points, W1, b1, W2, b2, Wp, bp, gamma, beta, rmean, rvar):
    points = _f32(points)
    inv_mx = np.float32(1.0) / MX

    def dev_q(u):
        return (u + np.float32(50.0)) * inv_mx

    def ref_q(u):
        return (u + np.float32(50.0)) / MX

    fixed = [None] * B
    for b_ in range(B):
        pf = points[b_].copy()
        for col in (0, 1):
            u = pf[:, col]
            qd, qr = dev_q(u), ref_q(u)
            bad = np.floor(qd) != np.floor(qr)
            if bad.any():
                kr = np.floor(qr[bad])
                ctr = (kr + np.float32(0.5)) * MX - np.float32(50.0)
                oob = (qr[bad] < 0) | (qr[bad] >= 128)
                u[bad] = np.where(oob, np.float32(1e4),
                                  ctr.astype(np.float32))
        fixed[b_] = pf

    in_maps = []
    for c_ in range(NCORES):
        b_, h_ = c_ // 2, c_ % 2
        pf = fixed[b_]
        x, y = pf[:, 0], pf[:, 1]
        xq, yq = dev_q(x), dev_q(y)
        valid = (xq >= 0) & (xq < 128) & (yq >= 0) & (yq < 128)
        ix = np.floor(xq).astype(np.int64)
        iy = np.floor(yq).astype(np.int64)
        g = iy * 128 + ix
        own = valid & ((g >= HALF) == bool(h_))
        tidx = np.nonzero(own)[0]
        n = tidx.shape[0]
        assert n <= TOK, f"core {c_}: {n} tokens > {TOK}"

        gl = g[tidx] - HALF * h_
        trow = gl >> 1
        tpar = gl & 1
        rid = _assign_rounds(trow, tpar, np.ones(gl.shape[0], dtype=bool))

        # slot assignment: sort tokens by round (stable), sequential slots
        order = np.argsort(rid, kind="stable")
        rs = rid[order]
        counts = np.bincount(rs, minlength=len(ROUNDS))
        caps = np.array([s for s, _ in ROUNDS])
        assert (counts <= caps).all(), \
            f"core {c_}: round overflow {counts.max()} vs {caps.min()}"
        within = np.arange(n) - np.concatenate(
            [[0], np.cumsum(counts)])[rs]
        pos = ROUND_OFF[rs] + within          # global stream slot per token

        arr = np.empty((TOK, 5), dtype=np.float32)
        arr[:, 0] = 1e4
        arr[:, 1] = 1e4
        arr[:, 2] = 0.0
        arr[:, 3] = 0.0
        arr[:, 4] = 1.0
        arr[pos, 0:4] = pf[tidx[order]]

        ptsb = np.empty(5 * TOK, dtype=np.float32)
        for rp in range(0, len(ROUNDS), 2):
            o = int(ROUND_OFF[rp])
            s2 = sum(s_ for s_, _ in ROUNDS[rp:rp + 2])
            ptsb[5 * o:5 * (o + s2)] = arr[o:o + s2, :].T.ravel()
        xw = np.ascontiguousarray(arr[:, 0].reshape(NW, 128).T)
        yw = np.ascontiguousarray(arr[:, 1].reshape(NW, 128).T)
        hb = np.full((128, 1), HALF * h_, dtype=np.float32)
        in_maps.append({"ptsb": ptsb, "xw": xw, "yw": yw, "hbase": hb})

    w1ext = np.zeros((5, ES), dtype=np.float32)
    w1ext[0:4, 0:C] = _f32(W1)
    w1ext[4, 0:C] = _f32(b1)
    w1ext[4, C] = 1.0
    w2ext = np.zeros((ES, ES), dtype=np.float32)
    w2ext[0:C, 0:C] = _f32(W2)
    w2ext[C, 0:C] = _f32(b2)
    w2ext[C, C] = 1.0
    wpext = np.zeros((ES, COUT), dtype=np.float32)
    wpext[0:C, :] = _f32(Wp)
    wpext[C, :] = _f32(bp)
    scale = _f32(gamma) / np.sqrt(_f32(rvar) + np.float32(1e-5))
    shift = _f32(beta) - _f32(rmean) * scale
    shared = {
        "w1ext": w1ext, "w2ext": w2ext, "wpext": wpext,
        "bnsc": _f32(scale).reshape(128, 1),
        "bnsh": _f32(shift).reshape(128, 1),
    }
    for m in in_maps:
        m.update(shared)
    return in_maps


# device column c of a [COUT, HALF] quarter holds bucket:
#   t = c // 128; k = c % 128; bucket = 2*(t*64 + k%64) + k//64
_t = np.arange(HALF) // 128
_k = np.arange(HALF) % 128
DEV_COL_BUCKET = 2 * (_t * 64 + _k % 64) + _k // 64
UNPERM = np.argsort(DEV_COL_BUCKET)          # bucket -> device column


def kernel(points, W1, b1, W2, b2, Wp, bp, gamma, beta, rmean, rvar,
           _trace=False):
    from concourse.bass_utils import run_bass_kernel_spmd

    if "prog" not in _PROG_CACHE:
        _PROG_CACHE["prog"] = _build_program()
    nc = _PROG_CACHE["prog"]

    in_maps = _host_prep(points, W1, b1, W2, b2, Wp, bp, gamma, beta,
                         rmean, rvar)
    res = run_bass_kernel_spmd(nc, in_maps, list(range(NCORES)),
                               trace=_trace)
    out = np.empty((B, COUT, HW), dtype=np.float32)
    for c_ in range(NCORES):
        b_, h_ = c_ // 2, c_ % 2
        quarter = res.results[c_]["out"]          # [COUT, HALF], permuted cols
        out[b_, :, h_ * HALF:(h_ + 1) * HALF] = quarter[:, UNPERM]
    out = out.reshape(B, COUT, BEV_H, BEV_W)
    if _trace:
        return out, res
    return out


# revision 20
# speedup vs baseline: 1.0988x; 1.0988x over previous
"""PointsToBEV Trainium2 kernel.

Sharding: frame b = core//2; each core of a pair owns half the BEV grid
(buckets [0,8192) even cores, [8192,16384) odd). The host routes each point
to the core owning its bucket, so no cross-core reduction is needed.

Scatter strategy: gpsimd.dma_scatter_add loses updates when two descriptors
for the same destination row are in flight concurrently (measured on HW), but
is exact when indices are unique within an instruction and instructions are
serialized via their DMA-completion edge. The host therefore assigns each
bucket's tokens to distinct "rounds" (one scatter instruction each). Rounds
alternate between two parity-split accumulators (bucket%2) so consecutive
rounds touch disjoint tensors and their transfers overlap, while same-buffer
rounds serialize through Tile's WAW edge. Pad slots hold sentinel points that
the device routes to the dummy row (4096); invalid points are dropped on the
host since the reference discards their bucket entirely.

Device pipeline per round: 2-layer point MLP on PE (biases folded via
constant-1 feature rows; count folded as emb channel 80), DVE relu-copy into
the scatter source, one dma_scatter_add of 81-float rows at 512B stride into
acc_E/acc_O [4097, 128] (row 4096 = dummy for invalid/out-of-half points).

Phase 2 per 128-bucket tile (64 even rows stacked over 64 odd rows):
mean = sums * recip(max(cnt,1)), PE transpose, 1x1 conv as matmul with bias
row, BN+ReLU via ACT scale/bias, DMA out. The host un-permutes the stacked
column order when assembling the final (B, 128, 128, 128) output.
"""

import numpy as np

BEV_H, BEV_W = 128, 128
HW = BEV_H * BEV_W
HALF = HW // 2              # buckets per core
QROW = HALF // 2            # rows per parity accumulator (4096)
X_MIN, Y_MIN = -50.0, -50.0
MX = np.float32(0.78125)
B, NP_, C, COUT = 4, 200000, 80, 128
NCORES = 8

# round schedule: (slots, parity) — shared by host and device.
# Invalid points are dropped on the host (they cannot affect the output);
# pad slots use sentinel points that the device maps to the dummy row.
N_BIG, S_BIG = 36, 2560
N_TAIL, S_TAIL = 24, 128
ROUNDS = [(S_BIG, j % 2) for j in range(N_BIG)] + \
         [(S_TAIL, j % 2) for j in range(N_TAIL)]
ROUND_OFF = np.concatenate([[0], np.cumsum([s for s, _ in ROUNDS])])
TOK = int(ROUND_OFF[-1])    # 95232
NW = TOK // 128             # 960
NIX = TOK // 16             # 7680

ES = C + 1                  # 81 floats per scattered row
ESTEP = 128                 # row stride (512B, multiple of 256B)
NROW = QROW + 1             # 4097 rows per accumulator (last = dummy)

_PROG_CACHE = {}
USE_FP32R = True


def _f32(x):
    return np.ascontiguousarray(x, dtype=np.float32)


def _build_program(dbg=False):
    from concourse import bacc, mybir, tile
    from concourse.masks import make_identity

    fp32 = mybir.dt.float32
    i16 = mybir.dt.int16
    i32 = mybir.dt.int32

    nc = bacc.Bacc(
        None,
        target_bir_lowering=False,
        debug=False,
        num_devices=NCORES,
        num_swdge_queues=4,
    )

    fp32r = mybir.dt.float32r if USE_FP32R else fp32
    ptsb = nc.dram_tensor("ptsb", [5 * TOK], fp32r, kind="ExternalInput")
    xw_d = nc.dram_tensor("xw", [128, NW], fp32, kind="ExternalInput")
    yw_d = nc.dram_tensor("yw", [128, NW], fp32, kind="ExternalInput")
    hb_d = nc.dram_tensor("hbase", [128, 1], fp32, kind="ExternalInput")
    w1_d = nc.dram_tensor("w1ext", [5, ES], fp32r, kind="ExternalInput")
    w2_d = nc.dram_tensor("w2ext", [ES, ES], fp32, kind="ExternalInput")
    wp_d = nc.dram_tensor("wpext", [ES, COUT], fp32, kind="ExternalInput")
    bnsc_d = nc.dram_tensor("bnsc", [128, 1], fp32, kind="ExternalInput")
    bnsh_d = nc.dram_tensor("bnsh", [128, 1], fp32, kind="ExternalInput")
    outp = nc.dram_tensor("out", [COUT, HALF], fp32, kind="ExternalOutput")
    ackind = "ExternalOutput" if dbg else "Internal"
    acc = [nc.dram_tensor(f"acc{p}", [NROW, ESTEP], fp32, kind=ackind)
           for p in (0, 1)]
    dbg_idx = (nc.dram_tensor("dbgidx", [128, NIX], i16, kind="ExternalOutput")
               if dbg else None)

    with tile.TileContext(nc) as tc:
        with tc.tile_pool(name="consts", bufs=1) as cp:
            w1t = cp.tile([5, ES], fp32r)
            nc.sync.dma_start(out=w1t[:], in_=w1_d[:])
            w2t = cp.tile([ES, ES], fp32)
            nc.sync.dma_start(out=w2t[:], in_=w2_d[:])
            wpt = cp.tile([ES, COUT], fp32)
            nc.sync.dma_start(out=wpt[:], in_=wp_d[:])
            bnsc = cp.tile([128, 1], fp32)
            nc.sync.dma_start(out=bnsc[:], in_=bnsc_d[:])
            bnsh = cp.tile([128, 1], fp32)
            nc.sync.dma_start(out=bnsh[:], in_=bnsh_d[:])
            hbt = cp.tile([128, 1], fp32)
            nc.sync.dma_start(out=hbt[:], in_=hb_d[:])
            ident = cp.tile([128, 128], fp32)
            make_identity(nc, ident[:])
            idxrep = cp.tile([128, NIX], i16)

            with tc.tile_pool(name="idxp", bufs=1) as ip:
                # zero both accumulators (rows [0:4096]; dummy row skipped)
                zt = ip.tile([128, 2048], fp32)
                nc.vector.memset(zt[:], 0.0)
                for p in (0, 1):
                    for k in range(2):
                        nc.sync.dma_start(
                            out=acc[p][k * 2048:(k + 1) * 2048, :], in_=zt[:]
                        )

                # ---- index computation (whole core, [128, NW]) ----
                xwt = ip.tile([128, NW], fp32)
                nc.sync.dma_start(out=xwt[:], in_=xw_d[:])
                ywt = ip.tile([128, NW], fp32)
                nc.sync.dma_start(out=ywt[:], in_=yw_d[:])

                # q = (u - MIN) * fl(1/MX); host pre-nudges points where this
                # disagrees with the reference's IEEE division.
                INV_MX = float(np.float32(1.0) / MX)
                xq = ip.tile([128, NW], fp32)
                nc.vector.tensor_scalar(
                    xq[:], xwt[:], -X_MIN, INV_MX,
                    op0=mybir.AluOpType.add, op1=mybir.AluOpType.mult,
                )
                yq = ip.tile([128, NW], fp32)
                nc.vector.tensor_scalar(
                    yq[:], ywt[:], -Y_MIN, INV_MX,
                    op0=mybir.AluOpType.add, op1=mybir.AluOpType.mult,
                )

                # floor(q) exactly, independent of int-cast rounding mode:
                # k0 = cast(q) within +-1 of floor; k1 = k0 + (q >= k0+1);
                # k = k1 - (q < k1).
                ti = ip.tile([128, NW], i32)
                tp1 = ip.tile([128, NW], fp32)
                ta = ip.tile([128, NW], fp32)

                def floor_exact(out_f, q_ap):
                    nc.vector.tensor_copy(ti[:], q_ap)
                    nc.vector.tensor_copy(out_f[:], ti[:])
                    nc.vector.tensor_scalar(
                        tp1[:], out_f[:], 1.0, None, op0=mybir.AluOpType.add
                    )
                    nc.vector.tensor_tensor(
                        out=ta[:], in0=q_ap, in1=tp1[:],
                        op=mybir.AluOpType.is_ge,
                    )
                    nc.vector.tensor_tensor(
                        out=out_f[:], in0=out_f[:], in1=ta[:],
                        op=mybir.AluOpType.add,
                    )
                    nc.vector.tensor_tensor(
                        out=ta[:], in0=q_ap, in1=out_f[:],
                        op=mybir.AluOpType.is_lt,
                    )
                    nc.vector.tensor_tensor(
                        out=out_f[:], in0=out_f[:], in1=ta[:],
                        op=mybir.AluOpType.subtract,
                    )

                ixf = ip.tile([128, NW], fp32)
                floor_exact(ixf, xq[:])
                iyf = ip.tile([128, NW], fp32)
                floor_exact(iyf, yq[:])

                # g_local = iy*128 + ix - halfbase
                g = ip.tile([128, NW], fp32)
                nc.vector.tensor_scalar(
                    g[:], iyf[:], 128.0, None, op0=mybir.AluOpType.mult
                )
                nc.vector.tensor_tensor(
                    out=g[:], in0=g[:], in1=ixf[:], op=mybir.AluOpType.add
                )
                nc.vector.tensor_scalar(
                    g[:], g[:], hbt[:, 0:1], None, op0=mybir.AluOpType.subtract
                )
                # validity: in-grid and in-half
                v = ip.tile([128, NW], fp32)
                t2 = ip.tile([128, NW], fp32)
                nc.vector.tensor_scalar(
                    v[:], xq[:], 0.0, None, op0=mybir.AluOpType.is_ge
                )
                for src_ap, thr, op in (
                    (xq, 128.0, mybir.AluOpType.is_lt),
                    (yq, 0.0, mybir.AluOpType.is_ge),
                    (yq, 128.0, mybir.AluOpType.is_lt),
                    (g, 0.0, mybir.AluOpType.is_ge),
                    (g, float(HALF), mybir.AluOpType.is_lt),
                ):
                    nc.vector.tensor_scalar(
                        t2[:], src_ap[:], thr, None, op0=op
                    )
                    nc.vector.tensor_tensor(
                        out=v[:], in0=v[:], in1=t2[:], op=mybir.AluOpType.mult
                    )
                # select: g = (g - HALF)*v + HALF  (invalid -> dummy 8192)
                nc.vector.tensor_scalar(
                    g[:], g[:], float(HALF), None, op0=mybir.AluOpType.subtract
                )
                nc.vector.tensor_tensor(
                    out=g[:], in0=g[:], in1=v[:], op=mybir.AluOpType.mult
                )
                nc.vector.tensor_scalar(
                    g[:], g[:], float(HALF), None, op0=mybir.AluOpType.add
                )
                # row = floor(g * 0.5) in [0, 4096]; 4096 = dummy.
                # NB: must not pass tp1 as q_ap — floor_exact writes tp1.
                gh = ip.tile([128, NW], fp32)
                nc.vector.tensor_scalar(
                    gh[:], g[:], 0.5, None, op0=mybir.AluOpType.mult
                )
                row = ip.tile([128, NW], fp32)
                floor_exact(row, gh[:])
                # int16 cast (aligned), then shuffle to the idx wrap via
                # SBUF->SBUF DMA (engine ops need 32-aligned partition bases)
                g16 = ip.tile([128, NW], i16)
                nc.vector.tensor_copy(g16[:], row[:])
                for q in range(8):
                    nc.sync.dma_start(
                        out=idxrep[0:16, q:NIX:8],
                        in_=g16[16 * q:16 * q + 16, :],
                    )
                nc.sync.dma_start(out=idxrep[16:32, :], in_=idxrep[0:16, :])
                nc.sync.dma_start(out=idxrep[32:64, :], in_=idxrep[0:32, :])
                nc.sync.dma_start(out=idxrep[64:128, :], in_=idxrep[0:64, :])
                if dbg:
                    nc.sync.dma_start(out=dbg_idx[:, :], in_=idxrep[:])

            # ---- phase 1: MLP + scatter rounds ----
            with (
                tc.tile_pool(name="pts", bufs=3) as pp,
                tc.tile_pool(name="ht", bufs=3) as hp,
                tc.tile_pool(name="src", bufs=2) as sp,
                tc.tile_pool(name="ps1", bufs=2, space="PSUM") as ps1,
                tc.tile_pool(name="ps2", bufs=3, space="PSUM") as ps2,
            ):
                for rp in range(0, len(ROUNDS), 2):
                  pair = [(rj, ROUNDS[rj]) for rj in (rp, rp + 1)
                          if rj < len(ROUNDS)]
                  off0 = int(ROUND_OFF[rp])
                  stot = sum(s_ for _, (s_, _) in pair)
                  ptst = pp.tile([5, 2 * S_BIG], fp32r, tag="pts")
                  nc.sync.dma_start(
                      out=ptst[:, 0:stot],
                      in_=ptsb[5 * off0:5 * (off0 + stot)],
                  )
                  for rj, (S, par) in pair:
                    off = int(ROUND_OFF[rj])
                    rel = off - off0
                    srct = sp.tile([128, (S_BIG // 128) * ES], fp32, tag="src")
                    done = 0
                    while done < S:
                        # up to 1024 tokens per group: 2 mm1 matmuls into one
                        # 2-bank psum tile, ONE relu, 8 mm2, 2 relu-copies
                        gt = min(1024, S - done)
                        p1 = ps1.tile([ES, 1024], fp32, tag="p1")
                        for h in range(0, gt, 512):
                            nt = min(512, gt - h)
                            nc.tensor.matmul(
                                p1[:, h:h + nt],
                                lhsT=w1t[:],
                                rhs=ptst[:, rel + done + h:rel + done + h + nt],
                                start=True, stop=True,
                            )
                        ht = hp.tile([ES, 1024], fp32, tag="ht")
                        nc.scalar.activation(
                            ht[:, 0:gt], p1[:, 0:gt],
                            mybir.ActivationFunctionType.Relu,
                        )
                        for half in range(0, gt, 512):
                            nt = min(512, gt - half)
                            p2 = ps2.tile([128, 4 * ES], fp32, tag="p2")
                            nm = nt // 128
                            for m in range(nm):
                                nc.tensor.matmul(
                                    p2[:, m * ES:(m + 1) * ES],
                                    lhsT=ht[:, half + m * 128:
                                            half + (m + 1) * 128],
                                    rhs=w2t[:],
                                    start=True, stop=True,
                                )
                            nc.vector.tensor_scalar_max(
                                srct[:, ((done + half) // 128) * ES:
                                     ((done + half) // 128 + nm) * ES],
                                p2[:, 0:nm * ES], 0.0,
                            )
                        done += gt
                    nc.gpsimd.dma_scatter_add(
                        out_ap=acc[par][:, 0:ES],
                        in_ap=srct[:, 0:(S // 128) * ES].rearrange(
                            "p (a b) -> p a b", b=ES
                        ),
                        idxs_ap=idxrep[:, off // 16:off // 16 + S // 16],
                        num_idxs=S,
                        num_idxs_reg=S,
                        elem_size=ES,
                        elem_step=ESTEP,
                        queue_num=rj % 4,
                    )

            # ---- phase 2: grouped 4-tile DMAs ----
            with (
                tc.tile_pool(name="p2s", bufs=3) as p2s,
                tc.tile_pool(name="p2m", bufs=3) as p2m,
                tc.tile_pool(name="pst", bufs=2, space="PSUM") as pst,
                tc.tile_pool(name="pso", bufs=2, space="PSUM") as pso,
            ):
                GT = 4                      # tiles per DMA group
                for tg in range(HALF // 128 // GT):
                    sbG = p2s.tile([128, GT, ESTEP], fp32, tag="sbG")
                    # bucket 2k   -> acc_E row k -> partitions 0..63
                    # bucket 2k+1 -> acc_O row k -> partitions 64..127
                    for par in (0, 1):
                        nc.sync.dma_start(
                            out=sbG[par * 64:par * 64 + 64, :, :],
                            in_=acc[par][tg * GT * 64:(tg + 1) * GT * 64, :]
                            .rearrange("(j p) c -> p j c", p=64),
                        )
                    ob4 = p2s.tile([COUT, GT * 128], fp32, tag="ob4")
                    for j in range(GT):
                        cm = p2m.tile([128, 1], fp32, tag="cm")
                        rc = p2m.tile([128, 1], fp32, tag="rc")
                        mt = p2m.tile([128, ES], fp32, tag="mt")
                        nc.vector.tensor_scalar_max(
                            cm[:], sbG[:, j, C:C + 1], 1.0
                        )
                        nc.vector.reciprocal(rc[:], cm[:])
                        nc.vector.tensor_scalar(
                            mt[:], sbG[:, j, 0:ES], rc[:, 0:1], None,
                            op0=mybir.AluOpType.mult,
                        )
                        nc.vector.memset(mt[:, C:C + 1], 1.0)
                        pt_ = pst.tile([ES, 128], fp32, tag="pt")
                        nc.tensor.transpose(pt_[:], mt[:], ident[:])
                        mtT = p2m.tile([ES, 128], fp32, tag="mtT")
                        nc.scalar.copy(mtT[:], pt_[:])
                        po = pso.tile([COUT, 128], fp32, tag="po")
                        nc.tensor.matmul(
                            po[:], lhsT=wpt[:], rhs=mtT[:],
                            start=True, stop=True,
                        )
                        nc.scalar.activation(
                            ob4[:, j * 128:(j + 1) * 128], po[:],
                            mybir.ActivationFunctionType.Relu,
                            bias=bnsh[:, 0:1], scale=bnsc[:, 0:1],
                        )
                    nc.sync.dma_start(
                        out=outp[:, tg * GT * 128:(tg + 1) * GT * 128],
                        in_=ob4[:],
                    )

    nc.compile()
    return nc


def _assign_rounds(row, parity, is_valid):
    """Assign each token to a round; same (parity,row) never repeats within a
    round. Returns per-token round id. row/parity only meaningful for valid."""
    n = row.shape[0]
    rid = np.empty(n, dtype=np.int64)
    big = [[j for j in range(N_BIG) if j % 2 == p] for p in (0, 1)]
    tail = [[N_BIG + j for j in range(N_TAIL) if j % 2 == p] for p in (0, 1)]
    # invalid tokens: dummy row duplicates are harmless -> spread round-robin
    inv_idx = np.nonzero(~is_valid)[0]
    rid[inv_idx] = np.arange(inv_idx.shape[0]) % N_BIG
    for p in (0, 1):
        sel = is_valid & (parity == p)
        idx = np.nonzero(sel)[0]
        if idx.size == 0:
            continue
        r = row[idx]
        order = np.argsort(r, kind="stable")
        rs = r[order]
        # occurrence number within bucket
        first = np.concatenate([[0], np.nonzero(np.diff(rs))[0] + 1])
        starts = np.zeros(rs.shape[0], dtype=np.int64)
        starts[first] = 1
        grp = np.cumsum(starts) - 1          # bucket enumeration id
        occ = np.arange(rs.shape[0]) - first[grp]
        nb, nt = len(big[p]), len(tail[p])
        assert (occ < nb + nt).all(), "bucket count exceeds round budget"
        rr = np.empty(rs.shape[0], dtype=np.int64)
        lo = occ < nb
        rr[lo] = np.array(big[p])[(grp[lo] + occ[lo]) % nb]
        if (~lo).any():
            rr[~lo] = np.array(tail[p])[(grp[~lo] + occ[~lo]) % nt]
        rid[idx[order]] = rr
    return rid


def _host_prep(points, W1, b1, W2, b2, Wp, bp, gamma, beta, rmean, rvar):
    points = _f32(points)
    inv_mx = np.float32(1.0) / MX

    def dev_q(u):
        return (u + np.float32(50.0)) * inv_mx

    def ref_q(u):
        return (u + np.float32(50.0)) / MX

    fixed = [None] * B
    for b_ in range(B):
        pf = points[b_].copy()
        for col in (0, 1):
            u = pf[:, col]
            qd, qr = dev_q(u), ref_q(u)
            bad = np.floor(qd) != np.floor(qr)
            if bad.any():
                kr = np.floor(qr[bad])
                ctr = (kr + np.float32(0.5)) * MX - np.float32(50.0)
                oob = (qr[bad] < 0) | (qr[bad] >= 128)
                u[bad] = np.where(oob, np.float32(1e4),
                                  ctr.astype(np.float32))
        fixed[b_] = pf

    in_maps = []
    for c_ in range(NCORES):
        b_, h_ = c_ // 2, c_ % 2
        pf = fixed[b_]
        x, y = pf[:, 0], pf[:, 1]
        xq, yq = dev_q(x), dev_q(y)
        valid = (xq >= 0) & (xq < 128) & (yq >= 0) & (yq < 128)
        ix = np.floor(xq).astype(np.int64)
        iy = np.floor(yq).astype(np.int64)
        g = iy * 128 + ix
        own = valid & ((g >= HALF) == bool(h_))
        tidx = np.nonzero(own)[0]
        n = tidx.shape[0]
        assert n <= TOK, f"core {c_}: {n} tokens > {TOK}"

        gl = g[tidx] - HALF * h_
        trow = gl >> 1
        tpar = gl & 1
        rid = _assign_rounds(trow, tpar, np.ones(gl.shape[0], dtype=bool))

        # slot assignment: sort tokens by round (stable), sequential slots
        order = np.argsort(rid, kind="stable")
        rs = rid[order]
        counts = np.bincount(rs, minlength=len(ROUNDS))
        caps = np.array([s for s, _ in ROUNDS])
        assert (counts <= caps).all(), \
            f"core {c_}: round overflow {counts.max()} vs {caps.min()}"
        within = np.arange(n) - np.concatenate(
            [[0], np.cumsum(counts)])[rs]
        pos = ROUND_OFF[rs] + within          # global stream slot per token

        arr = np.empty((TOK, 5), dtype=np.float32)
        arr[:, 0] = 1e4
        arr[:, 1] = 1e4
        arr[:, 2] = 0.0
        arr[:, 3] = 0.0
        arr[:, 4] = 1.0
        arr[pos, 0:4] = pf[tidx[order]]

        ptsb = np.empty(5 * TOK, dtype=np.float32)
        for rp in range(0, len(ROUNDS), 2):
            o = int(ROUND_OFF[rp])
            s2 = sum(s_ for s_, _ in ROUNDS[rp:rp + 2])
            ptsb[5 * o:5 * (o + s2)] = arr[o:o + s2, :].T.ravel()
        xw = np.ascontiguousarray(arr[:, 0].reshape(NW, 128).T)
        yw = np.ascontiguousarray(arr[:, 1].reshape(NW, 128).T)
        hb = np.full((128, 1), HALF * h_, dtype=np.float32)
        in_maps.append({"ptsb": ptsb, "xw": xw, "yw": yw, "hbase": hb})

    w1ext = np.zeros((5, ES), dtype=np.float32)
    w1ext[0:4, 0:C] = _f32(W1)
    w1ext[4, 0:C] = _f32(b1)
    w1ext[4, C] = 1.0
    w2ext = np.zeros((ES, ES), dtype=np.float32)
    w2ext[0:C, 0:C] = _f32(W2)
    w2ext[C, 0:C] = _f32(b2)
    w2ext[C, C] = 1.0
    wpext = np.zeros((ES, COUT), dtype=np.float32)
    wpext[0:C, :] = _f32(Wp)
    wpext[C, :] = _f32(bp)
    scale = _f32(gamma) / np.sqrt(_f32(rvar) + np.float32(1e-5))
    shift = _f32(beta) - _f32(rmean) * scale
    shared = {
        "w1ext": w1ext, "w2ext": w2ext, "wpext": wpext,
        "bnsc": _f32(scale).reshape(128, 1),
        "bnsh": _f32(shift).reshape(128, 1),
    }
    for m in in_maps:
        m.update(shared)
    return in_maps


# device column c of a [COUT, HALF] quarter holds bucket:
#   t = c // 128; k = c % 128; bucket = 2*(t*64 + k%64) + k//64
_t = np.arange(HALF) // 128
_k = np.arange(HALF) % 128
DEV_COL_BUCKET = 2 * (_t * 64 + _k % 64) + _k // 64
UNPERM = np.argsort(DEV_COL_BUCKET)          # bucket -> device column


def kernel(points, W1, b1, W2, b2, Wp, bp, gamma, beta, rmean, rvar,
           _trace=False):
    from concourse.bass_utils import run_bass_kernel_spmd

    if "prog" not in _PROG_CACHE:
        _PROG_CACHE["prog"] = _build_program()
    nc = _PROG_CACHE["prog"]

    in_maps = _host_prep(points, W1, b1, W2, b2, Wp, bp, gamma, beta,
                         rmean, rvar)
    res = run_bass_kernel_spmd(nc, in_maps, list(range(NCORES)),
                               trace=_trace)
    out = np.empty((B, COUT, HW), dtype=np.float32)
    for c_ in range(NCORES):
        b_, h_ = c_ // 2, c_ % 2
        quarter = res.results[c_]["out"]          # [COUT, HALF], permuted cols
        out[b_, :, h_ * HALF:(h_ + 1) * HALF] = quarter[:, UNPERM]
    out = out.reshape(B, COUT, BEV_H, BEV_W)
    if _trace:
        return out, res
    return out


# revision 21
# speedup vs baseline: 3463.0512x; 3151.7059x over previous
"""PointsToBEV Trainium2 kernel.

Sharding: frame b = core//2; each core of a pair owns half the BEV grid
(buckets [0,8192) even cores, [8192,16384) odd). The host routes each point
to the core owning its bucket, so no cross-core reduction is needed.

Scatter strategy: gpsimd.dma_scatter_add loses updates when two descriptors
for the same destination row are in flight concurrently (measured on HW), but
is exact when indices are unique within an instruction and instructions are
serialized via their DMA-completion edge. The host therefore assigns each
bucket's tokens to distinct "rounds" (one scatter instruction each). Rounds
alternate between two parity-split accumulators (bucket%2) so consecutive
rounds touch disjoint tensors and their transfers overlap, while same-buffer
rounds serialize through Tile's WAW edge. Pad slots hold sentinel points that
the device routes to the dummy row (4096); invalid points are dropped on the
host since the reference discards their bucket entirely.

Device pipeline per round: 2-layer point MLP on PE (biases folded via
constant-1 feature rows; count folded as emb channel 80), DVE relu-copy into
the scatter source, one dma_scatter_add of 81-float rows at 512B stride into
acc_E/acc_O [4097, 128] (row 4096 = dummy for invalid/out-of-half points).

Phase 2 per 128-bucket tile (64 even rows stacked over 64 odd rows):
mean = sums * recip(max(cnt,1)), PE transpose, 1x1 conv as matmul with bias
row, BN+ReLU via ACT scale/bias, DMA out. The host un-permutes the stacked
column order when assembling the final (B, 128, 128, 128) output.
"""

import numpy as np

BEV_H, BEV_W = 128, 128
HW = BEV_H * BEV_W
HALF = HW // 2              # buckets per core
QROW = HALF // 2            # rows per parity accumulator (4096)
X_MIN, Y_MIN = -50.0, -50.0
MX = np.float32(0.78125)
B, NP_, C, COUT = 4, 200000, 80, 128
NCORES = 8

# round schedule: (slots, parity) — shared by host and device.
# Invalid points are dropped on the host (they cannot affect the output);
# pad slots use sentinel points that the device maps to the dummy row.
N_BIG, S_BIG = 36, 2560
N_TAIL, S_TAIL = 24, 128
ROUNDS = [(S_BIG, j % 2) for j in range(N_BIG)] + \
         [(S_TAIL, j % 2) for j in range(N_TAIL)]
ROUND_OFF = np.concatenate([[0], np.cumsum([s for s, _ in ROUNDS])])
TOK = int(ROUND_OFF[-1])    # 95232
NW = TOK // 128             # 960
NIX = TOK // 16             # 7680

ES = C + 1                  # 81 floats per scattered row
ESTEP = 128                 # row stride (512B, multiple of 256B)
NROW = QROW + 1             # 4097 rows per accumulator (last = dummy)

_PROG_CACHE = {}
USE_FP32R = True


def _f32(x):
    return np.ascontiguousarray(x, dtype=np.float32)


def _build_program(dbg=False):
    from concourse import bacc, mybir, tile
    from concourse.masks import make_identity

    fp32 = mybir.dt.float32
    i16 = mybir.dt.int16
    i32 = mybir.dt.int32

    nc = bacc.Bacc(
        None,
        target_bir_lowering=False,
        debug=False,
        num_devices=NCORES,
        num_swdge_queues=4,
    )

    fp32r = mybir.dt.float32r if USE_FP32R else fp32
    ptsb = nc.dram_tensor("ptsb", [5 * TOK], fp32r, kind="ExternalInput")
    xw_d = nc.dram_tensor("xw", [128, NW], fp32, kind="ExternalInput")
    yw_d = nc.dram_tensor("yw", [128, NW], fp32, kind="ExternalInput")
    hb_d = nc.dram_tensor("hbase", [128, 1], fp32, kind="ExternalInput")
    w1_d = nc.dram_tensor("w1ext", [5, ES], fp32r, kind="ExternalInput")
    w2_d = nc.dram_tensor("w2ext", [ES, ES], fp32, kind="ExternalInput")
    wp_d = nc.dram_tensor("wpext", [ES, COUT], fp32, kind="ExternalInput")
    bnsc_d = nc.dram_tensor("bnsc", [128, 1], fp32, kind="ExternalInput")
    bnsh_d = nc.dram_tensor("bnsh", [128, 1], fp32, kind="ExternalInput")
    outp = nc.dram_tensor("out", [COUT, HALF], fp32, kind="ExternalOutput")
    ackind = "ExternalOutput" if dbg else "Internal"
    acc = [nc.dram_tensor(f"acc{p}", [NROW, ESTEP], fp32, kind=ackind)
           for p in (0, 1)]
    dbg_idx = (nc.dram_tensor("dbgidx", [128, NIX], i16, kind="ExternalOutput")
               if dbg else None)

    with tile.TileContext(nc) as tc:
        with tc.tile_pool(name="consts", bufs=1) as cp:
            w1t = cp.tile([5, ES], fp32r)
            nc.sync.dma_start(out=w1t[:], in_=w1_d[:])
            w2t = cp.tile([ES, ES], fp32)
            nc.sync.dma_start(out=w2t[:], in_=w2_d[:])
            wpt = cp.tile([ES, COUT], fp32)
            nc.sync.dma_start(out=wpt[:], in_=wp_d[:])
            bnsc = cp.tile([128, 1], fp32)
            nc.sync.dma_start(out=bnsc[:], in_=bnsc_d[:])
            bnsh = cp.tile([128, 1], fp32)
            nc.sync.dma_start(out=bnsh[:], in_=bnsh_d[:])
            hbt = cp.tile([128, 1], fp32)
            nc.sync.dma_start(out=hbt[:], in_=hb_d[:])
            ident = cp.tile([128, 128], fp32)
            make_identity(nc, ident[:])
            idxrep = cp.tile([128, NIX], i16)

            with tc.tile_pool(name="idxp", bufs=1) as ip:
                # zero both accumulators (rows [0:4096]; dummy row skipped)
                zt = ip.tile([128, 2048], fp32)
                nc.vector.memset(zt[:], 0.0)
                for p in (0, 1):
                    for k in range(2):
                        nc.sync.dma_start(
                            out=acc[p][k * 2048:(k + 1) * 2048, :], in_=zt[:]
                        )

                # ---- index computation (whole core, [128, NW]) ----
                xwt = ip.tile([128, NW], fp32)
                nc.sync.dma_start(out=xwt[:], in_=xw_d[:])
                ywt = ip.tile([128, NW], fp32)
                nc.sync.dma_start(out=ywt[:], in_=yw_d[:])

                # q = (u - MIN) * fl(1/MX); host pre-nudges points where this
                # disagrees with the reference's IEEE division.
                INV_MX = float(np.float32(1.0) / MX)
                xq = ip.tile([128, NW], fp32)
                nc.vector.tensor_scalar(
                    xq[:], xwt[:], -X_MIN, INV_MX,
                    op0=mybir.AluOpType.add, op1=mybir.AluOpType.mult,
                )
                yq = ip.tile([128, NW], fp32)
                nc.vector.tensor_scalar(
                    yq[:], ywt[:], -Y_MIN, INV_MX,
                    op0=mybir.AluOpType.add, op1=mybir.AluOpType.mult,
                )

                # floor(q) exactly, independent of int-cast rounding mode:
                # k0 = cast(q) within +-1 of floor; k1 = k0 + (q >= k0+1);
                # k = k1 - (q < k1).
                ti = ip.tile([128, NW], i32)
                tp1 = ip.tile([128, NW], fp32)
                ta = ip.tile([128, NW], fp32)

                def floor_exact(out_f, q_ap):
                    nc.vector.tensor_copy(ti[:], q_ap)
                    nc.vector.tensor_copy(out_f[:], ti[:])
                    nc.vector.tensor_scalar(
                        tp1[:], out_f[:], 1.0, None, op0=mybir.AluOpType.add
                    )
                    nc.vector.tensor_tensor(
                        out=ta[:], in0=q_ap, in1=tp1[:],
                        op=mybir.AluOpType.is_ge,
                    )
                    nc.vector.tensor_tensor(
                        out=out_f[:], in0=out_f[:], in1=ta[:],
                        op=mybir.AluOpType.add,
                    )
                    nc.vector.tensor_tensor(
                        out=ta[:], in0=q_ap, in1=out_f[:],
                        op=mybir.AluOpType.is_lt,
                    )
                    nc.vector.tensor_tensor(
                        out=out_f[:], in0=out_f[:], in1=ta[:],
                        op=mybir.AluOpType.subtract,
                    )

                ixf = ip.tile([128, NW], fp32)
                floor_exact(ixf, xq[:])
                iyf = ip.tile([128, NW], fp32)
                floor_exact(iyf, yq[:])

                # g_local = iy*128 + ix - halfbase
                g = ip.tile([128, NW], fp32)
                nc.vector.tensor_scalar(
                    g[:], iyf[:], 128.0, None, op0=mybir.AluOpType.mult
                )
                nc.vector.tensor_tensor(
                    out=g[:], in0=g[:], in1=ixf[:], op=mybir.AluOpType.add
                )
                nc.vector.tensor_scalar(
                    g[:], g[:], hbt[:, 0:1], None, op0=mybir.AluOpType.subtract
                )
                # validity: in-grid and in-half
                v = ip.tile([128, NW], fp32)
                t2 = ip.tile([128, NW], fp32)
                nc.vector.tensor_scalar(
                    v[:], xq[:], 0.0, None, op0=mybir.AluOpType.is_ge
                )
                for src_ap, thr, op in (
                    (xq, 128.0, mybir.AluOpType.is_lt),
                    (yq, 0.0, mybir.AluOpType.is_ge),
                    (yq, 128.0, mybir.AluOpType.is_lt),
                    (g, 0.0, mybir.AluOpType.is_ge),
                    (g, float(HALF), mybir.AluOpType.is_lt),
                ):
                    nc.vector.tensor_scalar(
                        t2[:], src_ap[:], thr, None, op0=op
                    )
                    nc.vector.tensor_tensor(
                        out=v[:], in0=v[:], in1=t2[:], op=mybir.AluOpType.mult
                    )
                # select: g = (g - HALF)*v + HALF  (invalid -> dummy 8192)
                nc.vector.tensor_scalar(
                    g[:], g[:], float(HALF), None, op0=mybir.AluOpType.subtract
                )
                nc.vector.tensor_tensor(
                    out=g[:], in0=g[:], in1=v[:], op=mybir.AluOpType.mult
                )
                nc.vector.tensor_scalar(
                    g[:], g[:], float(HALF), None, op0=mybir.AluOpType.add
                )
                # row = floor(g * 0.5) in [0, 4096]; 4096 = dummy.
                # NB: must not pass tp1 as q_ap — floor_exact writes tp1.
                gh = ip.tile([128, NW], fp32)
                nc.vector.tensor_scalar(
                    gh[:], g[:], 0.5, None, op0=mybir.AluOpType.mult
                )
                row = ip.tile([128, NW], fp32)
                floor_exact(row, gh[:])
                # int16 cast (aligned), then shuffle to the idx wrap via
                # SBUF->SBUF DMA (engine ops need 32-aligned partition bases)
                g16 = ip.tile([128, NW], i16)
                nc.vector.tensor_copy(g16[:], row[:])
                for q in range(8):
                    nc.sync.dma_start(
                        out=idxrep[0:16, q:NIX:8],
                        in_=g16[16 * q:16 * q + 16, :],
                    )
                nc.sync.dma_start(out=idxrep[16:32, :], in_=idxrep[0:16, :])
                nc.sync.dma_start(out=idxrep[32:64, :], in_=idxrep[0:32, :])
                nc.sync.dma_start(out=idxrep[64:128, :], in_=idxrep[0:64, :])
                if dbg:
                    nc.sync.dma_start(out=dbg_idx[:, :], in_=idxrep[:])

            # ---- phase 1: MLP + scatter rounds ----
            with (
                tc.tile_pool(name="pts", bufs=3) as pp,
                tc.tile_pool(name="ht", bufs=3) as hp,
                tc.tile_pool(name="src", bufs=2) as sp,
                tc.tile_pool(name="ps1", bufs=2, space="PSUM") as ps1,
                tc.tile_pool(name="ps2", bufs=3, space="PSUM") as ps2,
            ):
                for rp in range(0, len(ROUNDS), 2):
                  pair = [(rj, ROUNDS[rj]) for rj in (rp, rp + 1)
                          if rj < len(ROUNDS)]
                  off0 = int(ROUND_OFF[rp])
                  stot = sum(s_ for _, (s_, _) in pair)
                  ptst = pp.tile([5, 2 * S_BIG], fp32r, tag="pts")
                  nc.sync.dma_start(
                      out=ptst[:, 0:stot],
                      in_=ptsb[5 * off0:5 * (off0 + stot)],
                  )
                  for rj, (S, par) in pair:
                    off = int(ROUND_OFF[rj])
                    rel = off - off0
                    srct = sp.tile([128, (S_BIG // 128) * ES], fp32, tag="src")
                    done = 0
                    while done < S:
                        # up to 1024 tokens per group: 2 mm1 matmuls into one
                        # 2-bank psum tile, ONE relu, 8 mm2, 2 relu-copies
                        gt = min(1024, S - done)
                        p1 = ps1.tile([ES, 1024], fp32, tag="p1")
                        for h in range(0, gt, 512):
                            nt = min(512, gt - h)
                            nc.tensor.matmul(
                                p1[:, h:h + nt],
                                lhsT=w1t[:],
                                rhs=ptst[:, rel + done + h:rel + done + h + nt],
                                start=True, stop=True,
                            )
                        ht = hp.tile([ES, 1024], fp32, tag="ht")
                        nc.scalar.activation(
                            ht[:, 0:gt], p1[:, 0:gt],
                            mybir.ActivationFunctionType.Relu,
                        )
                        for half in range(0, gt, 512):
                            nt = min(512, gt - half)
                            p2 = ps2.tile([128, 4 * ES], fp32, tag="p2")
                            nm = nt // 128
                            for m in range(nm):
                                nc.tensor.matmul(
                                    p2[:, m * ES:(m + 1) * ES],
                                    lhsT=ht[:, half + m * 128:
                                            half + (m + 1) * 128],
                                    rhs=w2t[:],
                                    start=True, stop=True,
                                )
                            nc.vector.tensor_scalar_max(
                                srct[:, ((done + half) // 128) * ES:
                                     ((done + half) // 128 + nm) * ES],
                                p2[:, 0:nm * ES], 0.0,
                            )
                        done += gt
                    nc.gpsimd.dma_scatter_add(
                        out_ap=acc[par][:, 0:ES],
                        in_ap=srct[:, 0:(S // 128) * ES].rearrange(
                            "p (a b) -> p a b", b=ES
                        ),
                        idxs_ap=idxrep[:, off // 16:off // 16 + S // 16],
                        num_idxs=S,
                        num_idxs_reg=S,
                        elem_size=ES,
                        elem_step=ESTEP,
                        queue_num=rj % 4,
                    )

            # ---- phase 2: grouped 4-tile DMAs ----
            with (
                tc.tile_pool(name="p2s", bufs=3) as p2s,
                tc.tile_pool(name="p2m", bufs=3) as p2m,
                tc.tile_pool(name="pst", bufs=2, space="PSUM") as pst,
                tc.tile_pool(name="pso", bufs=2, space="PSUM") as pso,
            ):
                GT = 8                      # tiles per DMA group
                for tg in range(HALF // 128 // GT):
                    sbG = p2s.tile([128, GT, ESTEP], fp32, tag="sbG")
                    # bucket 2k   -> acc_E row k -> partitions 0..63
                    # bucket 2k+1 -> acc_O row k -> partitions 64..127
                    for par in (0, 1):
                        nc.sync.dma_start(
                            out=sbG[par * 64:par * 64 + 64, :, :],
                            in_=acc[par][tg * GT * 64:(tg + 1) * GT * 64, :]
                            .rearrange("(j p) c -> p j c", p=64),
                        )
                    ob4 = p2s.tile([COUT, GT * 128], fp32, tag="ob4")
                    for j in range(GT):
                        cm = p2m.tile([128, 1], fp32, tag="cm")
                        rc = p2m.tile([128, 1], fp32, tag="rc")
                        mt = p2m.tile([128, ES], fp32, tag="mt")
                        nc.vector.tensor_scalar_max(
                            cm[:], sbG[:, j, C:C + 1], 1.0
                        )
                        nc.vector.reciprocal(rc[:], cm[:])
                        nc.vector.tensor_scalar(
                            mt[:], sbG[:, j, 0:ES], rc[:, 0:1], None,
                            op0=mybir.AluOpType.mult,
                        )
                        nc.vector.memset(mt[:, C:C + 1], 1.0)
                        pt_ = pst.tile([ES, 128], fp32, tag="pt")
                        nc.tensor.transpose(pt_[:], mt[:], ident[:])
                        mtT = p2m.tile([ES, 128], fp32, tag="mtT")
                        nc.vector.tensor_copy(mtT[:], pt_[:])
                        po = pso.tile([COUT, 128], fp32, tag="po")
                        nc.tensor.matmul(
                            po[:], lhsT=wpt[:], rhs=mtT[:],
                            start=True, stop=True,
                        )
                        nc.scalar.activation(
                            ob4[:, j * 128:(j + 1) * 128], po[:],
                            mybir.ActivationFunctionType.Relu,
                            bias=bnsh[:, 0:1], scale=bnsc[:, 0:1],
                        )
                    nc.sync.dma_start(
                        out=outp[:, tg * GT * 128:(tg + 1) * GT * 128],
                        in_=ob4[:],
                    )

    nc.compile()
    return nc


def _assign_rounds(row, parity, is_valid):
    """Assign each token to a round; same (parity,row) never repeats within a
    round. Returns per-token round id. row/parity only meaningful for valid."""
    n = row.shape[0]
    rid = np.empty(n, dtype=np.int64)
    big = [[j for j in range(N_BIG) if j % 2 == p] for p in (0, 1)]
    tail = [[N_BIG + j for j in range(N_TAIL) if j % 2 == p] for p in (0, 1)]
    # invalid tokens: dummy row duplicates are harmless -> spread round-robin
    inv_idx = np.nonzero(~is_valid)[0]
    rid[inv_idx] = np.arange(inv_idx.shape[0]) % N_BIG
    for p in (0, 1):
        sel = is_valid & (parity == p)
        idx = np.nonzero(sel)[0]
        if idx.size == 0:
            continue
        r = row[idx]
        order = np.argsort(r, kind="stable")
        rs = r[order]
        # occurrence number within bucket
        first = np.concatenate([[0], np.nonzero(np.diff(rs))[0] + 1])
        starts = np.zeros(rs.shape[0], dtype=np.int64)
        starts[first] = 1
        grp = np.cumsum(starts) - 1          # bucket enumeration id
        occ = np.arange(rs.shape[0]) - first[grp]
        nb, nt = len(big[p]), len(tail[p])
        assert (occ < nb + nt).all(), "bucket count exceeds round budget"
        rr = np.empty(rs.shape[0], dtype=np.int64)
        lo = occ < nb
        rr[lo] = np.array(big[p])[(grp[lo] + occ[lo]) % nb]
        if (~lo).any():
            rr[~lo] = np.array(tail[p])[(grp[~lo] + occ[~lo]) % nt]
        rid[idx[order]] = rr
    return rid


def _host_prep(points, W1, b1, W2, b2, Wp, bp, gamma, beta, rmean, rvar):
    points = _f32(points)
    inv_mx = np.float32(1.0) / MX

    def dev_q(u):
        return (u + np.float32(50.0)) * inv_mx

    def ref_q(u):
        return (u + np.float32(50.0)) / MX

    fixed = [None] * B
    for b_ in range(B):
        pf = points[b_].copy()
        for col in (0, 1):
            u = pf[:, col]
            qd, qr = dev_q(u), ref_q(u)
            bad = np.floor(qd) != np.floor(qr)
            if bad.any():
                kr = np.floor(qr[bad])
                ctr = (kr + np.float32(0.5)) * MX - np.float32(50.0)
                oob = (qr[bad] < 0) | (qr[bad] >= 128)
                u[bad] = np.where(oob, np.float32(1e4),
                                  ctr.astype(np.float32))
        fixed[b_] = pf

    in_maps = []
    for c_ in range(NCORES):
        b_, h_ = c_ // 2, c_ % 2
        pf = fixed[b_]
        x, y = pf[:, 0], pf[:, 1]
        xq, yq = dev_q(x), dev_q(y)
        valid = (xq >= 0) & (xq < 128) & (yq >= 0) & (yq < 128)
        ix = np.floor(xq).astype(np.int64)
        iy = np.floor(yq).astype(np.int64)
        g = iy * 128 + ix
        own = valid & ((g >= HALF) == bool(h_))
        tidx = np.nonzero(own)[0]
        n = tidx.shape[0]
        assert n <= TOK, f"core {c_}: {n} tokens > {TOK}"

        gl = g[tidx] - HALF * h_
        trow = gl >> 1
        tpar = gl & 1
        rid = _assign_rounds(trow, tpar, np.ones(gl.shape[0], dtype=bool))

        # slot assignment: sort tokens by round (stable), sequential slots
        order = np.argsort(rid, kind="stable")
        rs = rid[order]
        counts = np.bincount(rs, minlength=len(ROUNDS))
        caps = np.array([s for s, _ in ROUNDS])
        assert (counts <= caps).all(), \
            f"core {c_}: round overflow {counts.max()} vs {caps.min()}"
        within = np.arange(n) - np.concatenate(
            [[0], np.cumsum(counts)])[rs]
        pos = ROUND_OFF[rs] + within          # global stream slot per token

        arr = np.empty((TOK, 5), dtype=np.float32)
        arr[:, 0] = 1e4
        arr[:, 1] = 1e4
        arr[:, 2] = 0.0
        arr[:, 3] = 0.0
        arr[:, 4] = 1.0
        arr[pos, 0:4] = pf[tidx[order]]

        ptsb = np.empty(5 * TOK, dtype=np.float32)
        for rp in range(0, len(ROUNDS), 2):
            o = int(ROUND_OFF[rp])
            s2 = sum(s_ for s_, _ in ROUNDS[rp:rp + 2])
            ptsb[5 * o:5 * (o + s2)] = arr[o:o + s2, :].T.ravel()
        xw = np.ascontiguousarray(arr[:, 0].reshape(NW, 128).T)
        yw = np.ascontiguousarray(arr[:, 1].reshape(NW, 128).T)
        hb = np.full((128, 1), HALF * h_, dtype=np.float32)
        in_maps.append({"ptsb": ptsb, "xw": xw, "yw": yw, "hbase": hb})

    w1ext = np.zeros((5, ES), dtype=np.float32)
    w1ext[0:4, 0:C] = _f32(W1)
    w1ext[4, 0:C] = _f32(b1)
    w1ext[4, C] = 1.0
    w2ext = np.zeros((ES, ES), dtype=np.float32)
    w2ext[0:C, 0:C] = _f32(W2)
    w2ext[C, 0:C] = _f32(b2)
    w2ext[C, C] = 1.0
    wpext = np.zeros((ES, COUT), dtype=np.float32)
    wpext[0:C, :] = _f32(Wp)
    wpext[C, :] = _f32(bp)
    scale = _f32(gamma) / np.sqrt(_f32(rvar) + np.float32(1e-5))
    shift = _f32(beta) - _f32(rmean) * scale
    shared = {
        "w1ext": w1ext, "w2ext": w2ext, "wpext": wpext,
        "bnsc": _f32(scale).reshape(128, 1),
        "bnsh": _f32(shift).reshape(128, 1),
    }
    for m in in_maps:
        m.update(shared)
    return in_maps


# device column c of a [COUT, HALF] quarter holds bucket:
#   t = c // 128; k = c % 128; bucket = 2*(t*64 + k%64) + k//64
_t = np.arange(HALF) // 128
_k = np.arange(HALF) % 128
DEV_COL_BUCKET = 2 * (_t * 64 + _k % 64) + _k // 64
UNPERM = np.argsort(DEV_COL_BUCKET)          # bucket -> device column


def kernel(points, W1, b1, W2, b2, Wp, bp, gamma, beta, rmean, rvar,
           _trace=False):
    from concourse.bass_utils import run_bass_kernel_spmd

    if "prog" not in _PROG_CACHE:
        _PROG_CACHE["prog"] = _build_program()
    nc = _PROG_CACHE["prog"]

    in_maps = _host_prep(points, W1, b1, W2, b2, Wp, bp, gamma, beta,
                         rmean, rvar)
    res = run_bass_kernel_spmd(nc, in_maps, list(range(NCORES)),
                               trace=_trace)
    out = np.empty((B, COUT, HW), dtype=np.float32)
    for c_ in range(NCORES):
        b_, h_ = c_ // 2, c_ % 2
        quarter = res.results[c_]["out"]          # [COUT, HALF], permuted cols
        out[b_, :, h_ * HALF:(h_ + 1) * HALF] = quarter[:, UNPERM]
    out = out.reshape(B, COUT, BEV_H, BEV_W)
    if _trace:
        return out, res
    return out
